# revision 1
# baseline (speedup 1.0000x reference)
"""DiffAttn kernel for 8 trn2 NeuronCores.

Problem (per reference):
  X [4, 4096, 1024]; Wq/Wk [1024, 256]; Wv [1024, 128]; biases; lam scalar.
  Q,K = X@Wq+bq, X@Wk+bk ; V = X@Wv+bv
  A_i = Q_i @ K_i^T / sqrt(128)  (i = 1,2 : the two 128-wide halves)
  out = (softmax(A1) - lam * softmax(A2)) @ V          -> [4, 4096, 128]

Sharding: 8 cores = 4 batches x 2 query-halves. Each core computes the
attention output for 2048 queries of one batch; K/V projections for the
full 4096 keys of that batch are computed redundantly on both cores of the
pair (no collectives). Host passes X^T per core with the core's query rows
ordered first; key order is irrelevant to softmax as long as K and V agree.

Per-core dataflow (compute matmuls in float32r: 4x fp32 throughput):
  XT [1024, 4096] streamed in 512-col chunks; Q/K projected into
  head-transposed layout [128, S]; V into [key, d] via PE transpose.
  Scores computed transposed S^T[sk, sq] over 1024-query super-chunks
  (per-component PSUM score slots); exp on ScalarE with 1/sqrt(D) folded
  into the activation scale; softmax denominators accumulated on VectorE;
  PV matmul with the V chunk stationary and E^T moving, accumulated over
  the 32 key tiles in PSUM. Finalize stays in the transposed layout: the
  denominator row is formed by an fp32 ones-matmul (column sum), inverted
  on VectorE, broadcast across partitions by a K=1 outer-product matmul,
  and applied with three tensor-tensor ops; the output ships as O^T
  [128, 2048] and the host transposes (pure layout).
"""

import os
import sys

sys.path.insert(0, "/opt/trn_rl_repo")

import numpy as np

import concourse.bacc as bacc
import concourse.mybir as mybir
from concourse import masks
from concourse.tile import TileContext
from concourse.bass_utils import run_bass_kernel_spmd

F32 = mybir.dt.float32
MM_F32R = os.environ.get("KERNEL_MM_DT", "f32r") == "f32r"
MM_DT = mybir.dt.float32r if MM_F32R else F32
AF = mybir.ActivationFunctionType

D = 128
EMB = 1024
B, S = 4, 4096
NQ = S // 2          # queries per core
SQC = 512            # projection column chunk
NCC = S // SQC       # 8 projection column chunks
NE = EMB // 128      # 8 emb tiles
SUP = 1024           # attention query super-chunk (2 psum banks)
NSUP = NQ // SUP     # 2
NSK = S // 128       # 32 key tiles
INV_SQRT_D = 1.0 / np.sqrt(np.float32(D))

TRACE = False
TRACE_DIR = None
LAST_RESULT = None


def _in(ap):
    """Bitcast a DRAM fp32 AP for DMA into an MM_DT tile."""
    return ap.bitcast(MM_DT) if MM_F32R else ap


def _f32(ap):
    return ap.bitcast(F32) if MM_F32R else ap


def _build():
    nc = bacc.Bacc("TRN2", target_bir_lowering=False, debug=False, num_devices=8)

    xt = nc.dram_tensor("xt", [EMB, S], F32, kind="ExternalInput")
    wq = nc.dram_tensor("wq", [EMB, 2 * D], F32, kind="ExternalInput")
    wk = nc.dram_tensor("wk", [EMB, 2 * D], F32, kind="ExternalInput")
    wv = nc.dram_tensor("wv", [EMB, D], F32, kind="ExternalInput")
    bq = nc.dram_tensor("bq", [2 * D, 1], F32, kind="ExternalInput")
    bk = nc.dram_tensor("bk", [2 * D, 1], F32, kind="ExternalInput")
    bv = nc.dram_tensor("bv", [D, 1], F32, kind="ExternalInput")
    lamv = nc.dram_tensor("lamv", [128, 1], F32, kind="ExternalInput")
    out = nc.dram_tensor("o", [D, NQ], F32, kind="ExternalOutput")  # O^T

    from contextlib import ExitStack

    with TileContext(nc) as tc, ExitStack() as ctx:
        xpool = ctx.enter_context(tc.tile_pool(name="xt", bufs=3))

        def load_chunk(cc, split=False):
            t = xpool.tile([128, NE, SQC], MM_DT, tag="xchunk", name=f"xc_{cc}")
            csl = slice(cc * SQC, (cc + 1) * SQC)
            if split:
                for e in range(NE):
                    nc.sync.dma_start(
                        out=t[:, e, :],
                        in_=_in(xt[e * 128 : (e + 1) * 128, csl]),
                    )
            else:
                nc.sync.dma_start(
                    out=t[:],
                    in_=_in(xt[:, csl]).rearrange("(t p) s -> p t s", p=128),
                )
            return t

        cpool = ctx.enter_context(tc.tile_pool(name="const", bufs=1))
        ident = cpool.tile([128, 128], F32)
        masks.make_identity(nc, ident[:])
        ones_col = cpool.tile([128, 1], F32, tag="ones_col")
        nc.vector.memset(ones_col[:], 1.0)
        ones_row = cpool.tile([1, 128], F32, tag="ones_row")
        nc.vector.memset(ones_row[:], 1.0)

        bq1 = cpool.tile([128, 1], F32, tag="bq1")
        bq2 = cpool.tile([128, 1], F32, tag="bq2")
        bk1 = cpool.tile([128, 1], F32, tag="bk1")
        bk2 = cpool.tile([128, 1], F32, tag="bk2")
        bvt = cpool.tile([128, 1], F32, tag="bvt")
        lam_t = cpool.tile([128, 1], F32, tag="lam")
        nc.gpsimd.dma_start(out=bq1[:], in_=bq[0:128, :])
        nc.gpsimd.dma_start(out=bq2[:], in_=bq[128:256, :])
        nc.gpsimd.dma_start(out=bk1[:], in_=bk[0:128, :])
        nc.gpsimd.dma_start(out=bk2[:], in_=bk[128:256, :])
        nc.gpsimd.dma_start(out=bvt[:], in_=bv[0:128, :])
        nc.gpsimd.dma_start(out=lam_t[:], in_=lamv[:, :])

        wpool = ctx.enter_context(tc.tile_pool(name="w", bufs=1))
        wq1 = wpool.tile([128, NE, 128], MM_DT, tag="wq1")
        wq2 = wpool.tile([128, NE, 128], MM_DT, tag="wq2")
        wk1 = wpool.tile([128, NE, 128], MM_DT, tag="wk1")
        wk2 = wpool.tile([128, NE, 128], MM_DT, tag="wk2")
        wvt = wpool.tile([128, NE, 128], MM_DT, tag="wvt")

        def wsrc(w, dsl):
            return _in(w[:, dsl]).rearrange("(t p) d -> p t d", p=128)

        # first projection group (k1) streams e-tile-wise: interleave its
        # weight slices with the first XT chunk so the PE starts ~4us in
        xt0 = xpool.tile([128, NE, SQC], MM_DT, tag="xchunk", name="xc_0")
        for e in range(NE):
            r = slice(e * 128, (e + 1) * 128)
            nc.sync.dma_start(out=wk1[:, e, :], in_=_in(wk[r, 0:128]))
            nc.sync.dma_start(out=xt0[:, e, :], in_=_in(xt[r, 0:512]))
        nc.sync.dma_start(out=wk2[:], in_=wsrc(wk, slice(128, 256)))
        nc.sync.dma_start(out=wvt[:], in_=wsrc(wv, slice(0, 128)))
        nc.sync.dma_start(out=wq1[:], in_=wsrc(wq, slice(0, 128)))
        nc.sync.dma_start(out=wq2[:], in_=wsrc(wq, slice(128, 256)))

        qkv = ctx.enter_context(tc.tile_pool(name="qkv", bufs=1))
        qt1 = qkv.tile([128, NQ], MM_DT, tag="qt1")
        qt2 = qkv.tile([128, NQ], MM_DT, tag="qt2")
        kt1 = qkv.tile([128, S], MM_DT, tag="kt1")
        kt2 = qkv.tile([128, S], MM_DT, tag="kt2")
        vv = qkv.tile([128, S], MM_DT, tag="vv")  # col c*128+j = V[key, d]

        epool = ctx.enter_context(tc.tile_pool(name="e", bufs=3))
        pspool = ctx.enter_context(tc.tile_pool(name="psums", bufs=2))
        fpool = ctx.enter_context(tc.tile_pool(name="fin", bufs=1))
        smpool = ctx.enter_context(tc.tile_pool(name="small", bufs=1))

        # ---------------- projections ----------------
        with ExitStack() as pctx:
            ppool = pctx.enter_context(tc.tile_pool(name="ppsum", bufs=1, space="PSUM"))
            tpool = pctx.enter_context(tc.tile_pool(name="ptr", bufs=2, space="PSUM"))
            vspool = pctx.enter_context(tc.tile_pool(name="vts", bufs=2))

            for cc in range(NCC):
                csl = slice(cc * SQC, (cc + 1) * SQC)
                xt_t = xt0 if cc == 0 else load_chunk(cc)

                groups = [(kt1, wk1, bk1, "k1"), (kt2, wk2, bk2, "k2")]
                if cc < NQ // SQC:
                    groups += [(qt1, wq1, bq1, "q1"), (qt2, wq2, bq2, "q2")]

                for dst, w_t, b_t, tag in groups:
                    ps = ppool.tile([128, SQC], F32, tag=tag, name=f"ps_{tag}_{cc}")
                    for t in range(NE):
                        nc.tensor.matmul(
                            ps[:],
                            w_t[:, t, :],
                            xt_t[:, t, :],
                            start=(t == 0),
                            stop=(t == NE - 1),
                        )
                    nc.scalar.activation(
                        dst[:, csl], ps[:], AF.Identity, bias=b_t[:, 0:1]
                    )

                # V^T chunk -> bias -> transpose into vv
                ps = ppool.tile([128, SQC], F32, tag="vt", name=f"ps_vt_{cc}")
                for t in range(NE):
                    nc.tensor.matmul(
                        ps[:],
                        wvt[:, t, :],
                        xt_t[:, t, :],
                        start=(t == 0),
                        stop=(t == NE - 1),
                    )
                vt_s = vspool.tile([128, SQC], F32, tag="vts")
                nc.scalar.activation(vt_s[:], ps[:], AF.Identity, bias=bvt[:, 0:1])
                for j in range(SQC // 128):
                    tr = tpool.tile([128, 128], F32, tag="vtr", name=f"vtr_{cc}_{j}")
                    nc.tensor.transpose(
                        tr[:], vt_s[:, j * 128 : (j + 1) * 128], ident[:]
                    )
                    col = (cc * (SQC // 128) + j) * 128
                    nc.vector.tensor_copy(vv[:, col : col + 128], tr[:])

        # ---------------- attention ----------------
        with ExitStack() as actx:
            spsum = actx.enter_context(tc.tile_pool(name="spsum", bufs=1, space="PSUM"))
            opsum = actx.enter_context(tc.tile_pool(name="opsum", bufs=1, space="PSUM"))

            st = {}

            def attn_state(c):
                st[c] = dict(
                    o1=opsum.tile([128, SUP], F32, tag="o1", name=f"o1_{c}"),
                    o2=opsum.tile([128, SUP], F32, tag="o2", name=f"o2_{c}"),
                    p1=pspool.tile([128, SUP], F32, tag="p1", name=f"p1_{c}"),
                    p2=pspool.tile([128, SUP], F32, tag="p2", name=f"p2_{c}"),
                )

            def attn_sk(c, skt):
                s = st[c]
                ksl = slice(skt * 128, (skt + 1) * 128)
                for comp, (ktc, qtc, pk, ok, etag) in enumerate(
                    [(kt1, qt1, "p1", "o1", "e1"), (kt2, qt2, "p2", "o2", "e2")]
                ):
                    s_ps = spsum.tile(
                        [128, SUP], F32, tag=f"s{comp}", name=f"s{comp}_{c}_{skt}"
                    )
                    for h in range(2):
                        hsl = slice(h * 512, (h + 1) * 512)
                        nc.tensor.matmul(
                            s_ps[:, hsl],
                            ktc[:, ksl],
                            qtc[:, c * SUP + h * 512 : c * SUP + (h + 1) * 512],
                            start=True,
                            stop=True,
                        )
                    e_t = epool.tile(
                        [128, SUP], MM_DT, tag=etag, name=f"{etag}_{c}_{skt}"
                    )
                    nc.scalar.activation(
                        e_t[:], s_ps[:], AF.Exp, scale=float(INV_SQRT_D)
                    )
                    pacc = s[pk]
                    if skt == 0:
                        nc.vector.tensor_copy(pacc[:], _f32(e_t[:]))
                    else:
                        nc.vector.tensor_add(pacc[:], pacc[:], _f32(e_t[:]))
                    o_ps = s[ok]
                    for h in range(2):
                        hsl = slice(h * 512, (h + 1) * 512)
                        nc.tensor.matmul(
                            o_ps[:, hsl],
                            vv[:, ksl],
                            e_t[:, hsl],
                            start=(skt == 0),
                            stop=(skt == NSK - 1),
                        )

            def finalize(c):
                s = st.pop(c)
                # evacuate O^T accumulators (frees the o psum slots fast)
                o1_s = fpool.tile([128, SUP], F32, tag="o1s", name=f"o1s_{c}")
                o2_s = fpool.tile([128, SUP], F32, tag="o2s", name=f"o2s_{c}")
                nc.scalar.copy(o1_s[:], s["o1"][:])
                nc.scalar.copy(o2_s[:], s["o2"][:])

                # denominator rows: rs[0, q] = sum_p pacc[p, q]  (fp32 matmul)
                invb = []
                for comp, (pk, stag) in enumerate([("p1", "s0"), ("p2", "s1")]):
                    rs = spsum.tile(
                        [1, SUP], F32, tag=stag, name=f"rs{comp}_{c}"
                    )
                    for h in range(2):
                        hsl = slice(h * 512, (h + 1) * 512)
                        nc.tensor.matmul(
                            rs[0:1, hsl],
                            ones_col[:],
                            s[pk][:, hsl],
                            start=True,
                            stop=True,
                        )
                    r = smpool.tile([1, SUP], F32, tag=f"r{comp}", name=f"r{comp}_{c}")
                    ib = spsum.tile(
                        [128, SUP], F32, tag=stag, name=f"ib{comp}_{c}"
                    )
                    for h in range(2):
                        hsl = slice(h * 512, (h + 1) * 512)
                        nc.vector.reciprocal(r[0:1, hsl], rs[0:1, hsl])
                        if comp == 1:
                            nc.vector.tensor_scalar_mul(
                                r[0:1, hsl], r[0:1, hsl], lam_t[0:1, 0:1]
                            )
                        nc.tensor.matmul(
                            ib[:, hsl],
                            ones_row[:],
                            r[0:1, hsl],
                            start=True,
                            stop=True,
                        )
                    invb.append(ib)

                t1 = fpool.tile([128, SUP], F32, tag="t1", name=f"t1_{c}")
                nc.vector.tensor_mul(t1[:], o1_s[:], invb[0][:])
                t2 = fpool.tile([128, SUP], F32, tag="t2", name=f"t2_{c}")
                nc.vector.tensor_mul(t2[:], o2_s[:], invb[1][:])
                o_t = fpool.tile([128, SUP], F32, tag="ot", name=f"ot_{c}")
                nc.vector.tensor_sub(o_t[:], t1[:], t2[:])
                nc.sync.dma_start(
                    out=out[:, c * SUP : (c + 1) * SUP], in_=o_t[:]
                )

            attn_state(0)
            for skt in range(NSK):
                attn_sk(0, skt)
            attn_state(1)
            for skt in range(12):
                attn_sk(1, skt)
            finalize(0)
            for skt in range(12, NSK):
                attn_sk(1, skt)
            finalize(1)

    nc.compile()
    return nc


_NC = None


def _get_nc():
    global _NC
    if _NC is None:
        _NC = _build()
    return _NC


def kernel(X, lam, Wq, bq, Wk, bk, Wv, bv):
    X = np.asarray(X, dtype=np.float32)
    lam_f = float(np.asarray(lam))
    Wq = np.ascontiguousarray(np.asarray(Wq, np.float32))
    Wk = np.ascontiguousarray(np.asarray(Wk, np.float32))
    Wv = np.ascontiguousarray(np.asarray(Wv, np.float32))
    bq_c = np.asarray(bq, np.float32).reshape(2 * D, 1).copy()
    bk_c = np.asarray(bk, np.float32).reshape(2 * D, 1).copy()
    bv_c = np.asarray(bv, np.float32).reshape(D, 1).copy()
    lam_v = np.full((128, 1), lam_f, np.float32)

    nc = _get_nc()

    in_maps = []
    for core in range(8):
        b, h = divmod(core, 2)
        xb = X[b]
        if h == 0:
            xr = xb
        else:
            xr = np.concatenate([xb[NQ:], xb[:NQ]], axis=0)
        xt_a = np.ascontiguousarray(xr.T)
        in_maps.append(
            {
                "xt": xt_a,
                "wq": Wq,
                "wk": Wk,
                "wv": Wv,
                "bq": bq_c,
                "bk": bk_c,
                "bv": bv_c,
                "lamv": lam_v,
            }
        )

    global LAST_RESULT
    kwargs = {}
    if TRACE:
        import tempfile

        tdir = tempfile.mkdtemp(dir=TRACE_DIR) if TRACE_DIR else None
        kwargs = dict(trace=True, tmpdir=tdir)
    res = run_bass_kernel_spmd(nc, in_maps, list(range(8)), **kwargs)
    LAST_RESULT = res

    o = np.empty((B, S, D), np.float32)
    for core in range(8):
        b, h = divmod(core, 2)
        o[b, h * NQ : (h + 1) * NQ, :] = res.results[core]["o"].T
    return o

